# revision 78
# baseline (speedup 1.0000x reference)
"""Trainium2 Bass kernel for nn_CNN_Casual (LeNet-ish CNN, B=8192).

Pure data parallel over 8 NeuronCores: 1024 samples per core, parameters
replicated, one SPMD Bass program. Per core, samples are processed in
blocks of 128 (the TensorEngine stationary-operand width), software-
pipelined three deep (conv1/pool1/T1 of block b overlaps evict/conv2/
pool2/T2 of block b-1 and the fc chain of block b-2):

  conv1  : host gathers x into overlapping windows (8 rows x 16 cols =
           K 128) and folds sigmoid(mask) into a per-window Toeplitz
           weight [128, 480] whose columns are PRE-SPLIT by pooling
           column parity (qr outermost), so pooling runs on contiguous
           APs; 12 fp16 matmuls per block into 1-bank PSUM tiles
           (a matmul must not write past its tile's first bank).
  pool1  : per tile, one of three statically balanced recipes (R/B/E,
           see the knob comments below); maxes on DVE at the packed-
           fp16 2x rate, copies on ScalarE.
  T1     : three xbar DMA transposes [128, 4x128] -> 4 chunks [120,128]
           each, issued as soon as their chunks are pooled (the DMA
           engines are otherwise ~30% busy; PE transposes and their
           PSUM round-trip are gone).
  relu+b1: fused into one dual-op tensor_scalar per [120, 512] chunk
           (add per-partition bias, max 0) - 4x DVE mode on fp16 SBUF,
           or the GPSIMD equivalent (tensor_scalar is the only compute
           GPSIMD can run here), per the static balance.
  conv2  : Toeplitz master [120, 7*160] = [Z,W4..W0,Z] fp16 (columns
           qr-pre-split like conv1); per output row-pair group, 6
           uniform full-width matmuls accumulate in PSUM.
  pool2/T2: same recipe split -> padded [128, 4x128]; one xbar DMA
           transpose -> f_t; relu+b2 via dual-op tensor_scalar [80,512].
  fc1    : weights stationary [80, 50] x 4 groups, moving = f chunks
           [80, 128]; relu+bias eviction on ScalarE -> fc1o [50, 128].
  fc2    : data stationary [50, 128], moving weights [50, 10]; DVE adds
           (fc2_b - 10) into t1_all (a constant per-sample shift is
           exact for log_softmax).
  softmax: ONE batched epilogue at the end (Exp, windowed reduce_sum,
           Ln, 8 per-partition-scalar subtracts, one output DMA), so the
           activation table is not reloaded mid-kernel.

dtypes: conv inputs/weights and pooled activations fp16 (end-to-end max
relative error ~4e-4 vs the fp32 reference); PSUM accumulation fp32.
"""

from contextlib import ExitStack

import numpy as np

import concourse.mybir as mybir
import concourse.tile as tile
from concourse import bacc
from concourse.bass_utils import run_bass_kernel_spmd

F32 = mybir.dt.float32
FP16 = mybir.dt.float16
AF = mybir.ActivationFunctionType
AX = mybir.AxisListType

N_CORES = 8
B_TOTAL = 8192
B_CORE = B_TOTAL // N_CORES  # 1024

# static engine balance knobs (tuned against the cost-model timeline).
# GPSIMD can only run tensor_scalar (the fused relu+bias evictions) on real
# hardware - no tensor_tensor - so all pooling maxes live on DVE/ScalarE:
# per conv1 tile (PSUM tiles are 1 bank each; a matmul may not write past
# its tile's first bank, and PSUM reads never cross banks):
#  "R": one direct DVE reduce (frees PSUM fastest, all DVE)
#  "B": ScalarE copy [480] frees PSUM; two packed-2x DVE maxes (A585+D307)
#  "E": ScalarE copies the qr=0 half, DVE maxes the qr=1 PSUM half against
#       it (one-PSUM-operand rule), then one packed rowmax (A385+D497)
POOL1_RECIPES = ["B", "B", "E", "R", "B", "E", "B", "R", "B", "B", "E", "R"]
POOL2_RECIPES = ["A", "B", "B", "B"]            # per conv2 group
EVICT1_ENGINES = ["gp", "gp", "dve"]            # x2cat chunks
EVICT2_ENGINE = "gp"                           # f_used


# --------------------------------------------------------------------------
# Host-side weight preparation (tiny tensors; exact rearrangement only)
# --------------------------------------------------------------------------
def prep_weights(mask_w, conv1_w, conv1_b, conv2_w, conv2_b, fc1_w, fc1_b,
                 fc2_w, fc2_b):
    f32 = np.float32
    sig = (1.0 / (1.0 + np.exp(-mask_w.astype(f32)))).astype(f32)  # [28,28]

    # conv1 Toeplitz windows with mask folded in.
    # window (w,h): input rows 4w..4w+7, cols 12h..12h+15 (K = 8*16 = 128)
    # col index of the moving matrix: dp*120 + o*12 + ql
    #   (output row p = 4w+dp, output col q = 12h+ql)
    # column order: qr*240 + u*120 + r*60 + o*6 + qh (dp = 2u+r,
    # ql = 2qh+qr), so the PSUM tile lands pre-split by pooling column -
    # every pooling op then runs on plain contiguous APs
    w1b = np.zeros((128, 480), f32)
    oo = np.arange(10)
    for dp in range(4):
        u, rr = dp // 2, dp % 2
        for ki in range(5):
            i = dp + ki
            for kj in range(5):
                for ql in range(12):
                    j = ql + kj
                    qh, qr = ql // 2, ql % 2
                    w1b[i * 16 + j,
                        qr * 240 + u * 120 + rr * 60 + oo * 6 + qh] = \
                        conv1_w[:, 0, ki, kj]
    w1m = np.empty((12, 128, 480), np.float16)
    for w in range(6):
        for h in range(2):
            win = sig[4 * w:4 * w + 8, 12 * h:12 * h + 16].reshape(128, 1)
            w1m[w * 2 + h] = (w1b * win).astype(np.float16)
    w1m = np.ascontiguousarray(w1m.transpose(1, 0, 2).reshape(128, 5760))

    # conv2 master Toeplitz: blocks [Z, W4, W3, W2, W1, W0, Z], each [120,160]
    # row index (c, j) = c*12 + j; col index qr*80 + o2*4 + qh (q2 = 2qh+qr,
    # same PSUM pre-split trick as conv1)
    w2m = np.zeros((120, 7, 160), np.float16)
    o2 = np.arange(20)
    for k in range(5):
        blk = 5 - k
        for c in range(10):
            for kj in range(5):
                for q2 in range(8):
                    j = q2 + kj
                    qh, qr = q2 // 2, q2 % 2
                    w2m[c * 12 + j, blk, qr * 80 + o2 * 4 + qh] = \
                        conv2_w[:, c, k, kj]
    w2m_flat = np.ascontiguousarray(w2m.reshape(120, 7 * 160))

    # fc1 weights per pooled-row group p': rows (o2, s2), torch flatten order
    # of the conv2 activations is (o2, p', s2).
    fc1w4 = fc1_w.reshape(50, 20, 4, 4)  # [m, o2, p', s2]
    wfc1 = np.concatenate(
        [np.ascontiguousarray(fc1w4[:, :, p, :].reshape(50, 80).T)
         for p in range(4)],
        axis=1,
    )  # [80, 200]

    # const blob 1 (fp32): bc2 | b1 | b2 | bf1  -> [128, 13]
    cst = np.zeros((128, 13), f32)
    # constant stabilizing shift for log_softmax (exact: any per-sample
    # constant cancels); logits stay well inside fp32 exp range
    cst[:, 0:10] = np.tile(fc2_b.astype(f32).reshape(1, 10) - 10.0, (128, 1))
    cst[0:120, 10] = np.repeat(conv1_b.astype(f32), 12)
    cst[0:80, 11] = np.repeat(conv2_b.astype(f32), 4)
    cst[0:50, 12] = fc1_b.astype(f32)

    # const blob 2 (fp16): fc2_w.T | wfc1 -> [80, 210]
    wfcb = np.zeros((80, 210), np.float16)
    wfcb[0:50, 0:10] = fc2_w.T.astype(np.float16)
    wfcb[:, 10:210] = wfc1.astype(np.float16)

    return dict(w1m=w1m, w2m=w2m_flat, wfcb=wfcb, cst=cst)


_prep_weights = prep_weights


# --------------------------------------------------------------------------
# Device program
# --------------------------------------------------------------------------
def _build(b_core):
    assert b_core % 256 == 0
    n_blk = b_core // 128

    nc = bacc.Bacc("TRN2", target_bir_lowering=False, debug=False,
                   num_devices=N_CORES)

    xw_d = nc.dram_tensor("xw", [12, 128, b_core], FP16,
                          kind="ExternalInput").ap()
    w1m_d = nc.dram_tensor("w1m", [128, 5760], FP16,
                           kind="ExternalInput").ap()
    w2m_d = nc.dram_tensor("w2m", [120, 1120], FP16, kind="ExternalInput").ap()
    wfcb_d = nc.dram_tensor("wfcb", [80, 210], FP16, kind="ExternalInput").ap()
    cst_d = nc.dram_tensor("cst", [128, 13], F32, kind="ExternalInput").ap()
    y = nc.dram_tensor("y", [b_core, 10], F32, kind="ExternalOutput").ap()

    MAX, ADD, SUB = (mybir.AluOpType.max, mybir.AluOpType.add,
                     mybir.AluOpType.subtract)

    with tile.TileContext(nc) as tc, ExitStack() as ctx:
        consts = ctx.enter_context(tc.tile_pool(name="consts", bufs=1))
        w1m_sb = consts.tile([128, 5760], FP16)
        w2m_sb = consts.tile([120, 1120], FP16)
        wfcb_sb = consts.tile([80, 210], FP16)
        cst_sb = consts.tile([128, 13], F32)

        bc2_sb = cst_sb[:, 0:10]
        b1_sb = cst_sb[0:120, 10:11]
        b2_sb = cst_sb[0:80, 11:12]
        bf1_sb = cst_sb[0:50, 12:13]
        wfc2_sb = wfcb_sb[0:50, 0:10]
        wfc1_sb = wfcb_sb[:, 10:210]

        # padded pooled layouts, static double buffers (pad cols memset once)
        pooled1 = [consts.tile([128, 1536], FP16, name=f"pooled1_{i}")
                   for i in range(2)]
        pooled2 = [consts.tile([128, 512], FP16, name=f"pooled2_{i}")
                   for i in range(2)]
        for t in pooled1:  # pad cols only; feature cols are fully written
            nc.vector.memset(
                t.rearrange("p (c f) -> p c f", c=12)[:, :, 120:128], 0.0)
        for t in pooled2:
            nc.vector.memset(
                t.rearrange("p (c f) -> p c f", c=4)[:, :, 80:128], 0.0)
        t1_all = consts.tile([128, 10 * n_blk], F32)

        xw_pool = ctx.enter_context(tc.tile_pool(name="xw", bufs=3))
        psb_pool = ctx.enter_context(tc.tile_pool(name="psb", bufs=3,
                                                  space="PSUM"))
        psr_pool = ctx.enter_context(tc.tile_pool(name="psr", bufs=2,
                                                  space="PSUM"))
        hc_pool = ctx.enter_context(tc.tile_pool(name="hc", bufs=4))
        rm_pool = ctx.enter_context(tc.tile_pool(name="rm", bufs=4))
        x2t_pool = ctx.enter_context(tc.tile_pool(name="x2t", bufs=6))
        x2c_pool = ctx.enter_context(tc.tile_pool(name="x2c", bufs=6))
        ps2_pool = ctx.enter_context(tc.tile_pool(name="ps2", bufs=2,
                                                  space="PSUM"))
        tm2_pool = ctx.enter_context(tc.tile_pool(name="tm2", bufs=4))
        ft_pool = ctx.enter_context(tc.tile_pool(name="ft", bufs=2))
        fu_pool = ctx.enter_context(tc.tile_pool(name="fu", bufs=2))
        fc1o_pool = ctx.enter_context(tc.tile_pool(name="fc1o", bufs=2))
        psf_pool = ctx.enter_context(tc.tile_pool(name="psf", bufs=1,
                                                  space="PSUM"))
        sm_pool = ctx.enter_context(tc.tile_pool(name="sm", bufs=1))

        def ts_relu_bias(eng, out, in_, bias):
            eng.tensor_scalar(out, in_, bias, 0.0, op0=ADD, op1=MAX)

        def w1m_load(t):
            eng = nc.sync if t % 2 == 0 else nc.scalar
            eng.dma_start(w1m_sb[:, t * 480:(t + 1) * 480],
                          w1m_d[:, t * 480:(t + 1) * 480])

        def conv1_mm(dst_ps, blk, xwcat, half, t):
            if blk == 0:
                w1m_load(t)
            nc.tensor.matmul(
                dst_ps,
                xwcat[:, t * 256 + half * 128:t * 256 + half * 128 + 128],
                w1m_sb[:, t * 480:(t + 1) * 480],
                start=True, stop=True)
            if blk == 0 and t == 2:
                nc.scalar.dma_start(cst_sb[:], cst_d)
                nc.sync.dma_start(w2m_sb[:], w2m_d)
                nc.scalar.dma_start(wfcb_sb[:], wfcb_d)

        def conv1_window(blk, xwcat, half, w):
            """conv1 + pool1 for one window (tiles 2w, 2w+1) of a block."""
            pl1 = pooled1[blk % 2]
            # pooled-chunk dst per h: chunk p' = 2w+u, feat = o*12 + 6h+qh
            # -> dims [u, o, qh] (3 free dims, qh packed)
            dv = (pl1.rearrange("p (w u f) -> p w u f", w=6, u=2)
                  [:, w, :, 0:120]
                  .rearrange("p u (o hh qh) -> p hh u o qh", o=10, hh=2))
            for h in range(2):
                t = 2 * w + h
                recipe = POOL1_RECIPES[t]
                pool = psr_pool if recipe == "R" else psb_pool
                ps1 = pool.tile([128, 480], F32, name="ps1_t",
                                tag="psr" if recipe == "R" else "psb")
                conv1_mm(ps1[:], blk, xwcat, half, t)
                if recipe == "R":
                    src = ps1.rearrange("p (qr u r o qh) -> p u o qh r qr",
                                        qr=2, u=2, r=2, o=10)
                    nc.vector.reduce_max(dv[:, h], src, axis=AX.XY)
                    continue
                rm = rm_pool.tile([128, 240], FP16, name="rm_t", tag="rm")
                if recipe == "B":
                    # copy frees PSUM; qr pre-split -> packed contiguous maxes
                    hc = hc_pool.tile([128, 480], FP16, name="hc_t", tag="hc")
                    nc.scalar.copy(hc[:], ps1[:])
                    nc.vector.tensor_tensor(rm[:], hc[:, 0:240],
                                            hc[:, 240:480], op=MAX)
                else:  # "E": half-copy + one-PSUM-operand TT
                    hc = hc_pool.tile([128, 240], FP16, name="hcE_t", tag="hc")
                    nc.scalar.copy(hc[:], ps1[:, 0:240])
                    nc.vector.tensor_tensor(rm[:], ps1[:, 240:480], hc[:],
                                            op=MAX)
                rv = rm.rearrange("p (u r o qh) -> p u r o qh",
                                  u=2, r=2, o=10)
                nc.vector.tensor_tensor(dv[:, h], rv[:, :, 0], rv[:, :, 1],
                                        op=MAX)

        def t1_part(blk, part):
            """xbar transpose of pooled chunks 4*part..4*part+3 (issued as
            soon as those four chunks are pooled)."""
            x2t = x2t_pool.tile([128, 512], FP16, name="x2t_b", tag="x2t")
            nc.sync.dma_start_transpose(
                x2t.rearrange("p (c f) -> p c f", c=4),
                pooled1[blk % 2][:, 512 * part:512 * part + 512])
            return x2t

        def conv2_evict(x2t, ww):
            """relu/bias on one landed x2t part -> one x2cat chunk."""
            x2c = x2c_pool.tile([120, 512], FP16, name="x2c_b",
                                tag=f"x2c{ww}")
            eng = {"dve": nc.vector, "gp": nc.gpsimd}[EVICT1_ENGINES[ww]]
            ts_relu_bias(eng, x2c[:], x2t[0:120, :], b1_sb)
            return x2c

        def conv2_group(blk, x2cat, g):
            """conv2 + pool2 for one output-row-pair group of a block."""
            pl2 = pooled2[blk % 2]
            B = [w2m_sb[:, k * 160:(k + 1) * 160] for k in range(7)]

            def lhs(r):
                return x2cat[r // 4][:, (r % 4) * 128:(r % 4 + 1) * 128]

            ps2 = ps2_pool.tile([128, 320], F32, name="ps2_g", tag="ps2")
            lo, hi = ps2[:, 0:160], ps2[:, 160:320]
            r = 2 * g
            # uniform 6-step accumulation (zero guard blocks keep every
            # matmul full width; split-region start/stop miscomputes on HW)
            del lo, hi
            for dd in range(6):
                nc.tensor.matmul(ps2[:], lhs(r + dd),
                                 w2m_sb[:, (5 - dd) * 160:(7 - dd) * 160],
                                 start=(dd == 0), stop=(dd == 5))
            # pool2: (pl, o, qh, qr) -> chunk g feat = o*4 + qh
            dst = (pl2.rearrange("p (g f) -> p g f", g=4)[:, g, 0:80]
                   .rearrange("p (o qh) -> p o qh", o=20))
            if POOL2_RECIPES[g] == "A":
                src = ps2.rearrange("p (pl qr o qh) -> p o qh pl qr",
                                    pl=2, qr=2, o=20)
                nc.vector.reduce_max(dst, src, axis=AX.XY)
            else:
                tm2 = tm2_pool.tile([128, 320], FP16, name="tm2_g",
                                    tag="tm2")
                nc.scalar.copy(tm2[:], ps2[:])
                tv = tm2.rearrange("p (pl qr f) -> p pl qr f", pl=2, qr=2)
                rm2 = tm2_pool.tile([128, 160], FP16, name="rm2_g",
                                    tag="rm2")
                nc.vector.tensor_tensor(rm2.rearrange("p (pl f) -> p pl f",
                                                      pl=2),
                                        tv[:, :, 0], tv[:, :, 1], op=MAX)
                r2 = rm2.rearrange("p (pl f) -> p pl f", pl=2)
                nc.vector.tensor_tensor(dst.rearrange("p o qh -> p (o qh)"),
                                        r2[:, 0], r2[:, 1], op=MAX)

        def t2_issue(blk):
            """xbar transpose of pooled2 -> f_t (issued one iteration after
            pool2 so the Act sequencer never blocks on it)."""
            f_t = ft_pool.tile([128, 512], FP16, name="f_t", tag="ft")
            nc.sync.dma_start_transpose(
                f_t.rearrange("p (c f) -> p c f", c=4), pooled2[blk % 2][:])
            return f_t

        def fc_front(blk, f_t):
            """relu/bias on the transposed features + fc1 matmuls."""
            f_u = fu_pool.tile([80, 512], FP16, name="f_u", tag="fu")
            eng = {"dve": nc.vector, "gp": nc.gpsimd}[EVICT2_ENGINE]
            ts_relu_bias(eng, f_u[:], f_t[0:80, :], b2_sb)
            psf1 = psf_pool.tile([50, 128], F32, name="psf1", tag="psf")
            for g in range(4):
                nc.tensor.matmul(psf1[:], wfc1_sb[:, g * 50:(g + 1) * 50],
                                 f_u[:, g * 128:(g + 1) * 128],
                                 start=(g == 0), stop=(g == 3))
            return psf1

        def fc_back(blk, psf1):
            """fc1 relu/bias eviction + fc2 + stabilized shift."""
            fc1o = fc1o_pool.tile([50, 128], FP16, name="fc1o", tag="fc1o")
            nc.scalar.activation(fc1o[:], psf1[:], AF.Relu, bias=bf1_sb)
            psf2 = psf_pool.tile([128, 10], F32, name="psf2", tag="psf")
            nc.tensor.matmul(psf2[:], fc1o[:], wfc2_sb, start=True, stop=True)
            nc.vector.tensor_tensor(t1_all[:, blk * 10:blk * 10 + 10],
                                    psf2[:], bc2_sb, op=ADD)

        def epilogue(b0, nb):
            """log_softmax + output DMA for blocks b0..b0+nb-1."""
            t1s = t1_all[:, b0 * 10:(b0 + nb) * 10]
            e_all = sm_pool.tile([128, 10 * nb], F32, name="e_all", tag="e")
            nc.scalar.activation(e_all[:], t1s, AF.Exp)
            se = sm_pool.tile([128, nb], F32, name="se", tag="se")
            nc.vector.reduce_sum(se[:],
                                 e_all.rearrange("p (b t) -> p b t", t=10),
                                 axis=AX.X)
            ls = sm_pool.tile([128, nb], F32, name="ls", tag="ls")
            nc.scalar.activation(ls[:], se[:], AF.Ln)
            yo = sm_pool.tile([128, 10 * nb], F32, name="yo", tag="yo")
            for b in range(nb):
                nc.vector.tensor_scalar(yo[:, b * 10:b * 10 + 10],
                                        t1s[:, b * 10:b * 10 + 10],
                                        ls[:, b:b + 1], None, op0=SUB)
            nc.sync.dma_start(
                y[b0 * 128:(b0 + nb) * 128]
                .rearrange("(blk p) c -> p blk c", p=128),
                yo.rearrange("p (blk c) -> p blk c", c=10))

        # ------------- software-pipelined main loop (depth 3) -------------
        # iteration it emits, interleaved per segment so every engine's
        # queue sees work in dependency-arrival order:
        #   evicts(it-1) | 4x[ conv1-tiles(it) + conv2-group(it-1) ] |
        #   T1-issue(it) | T2-issue(it-1) | fc-chain(it-2)
        def xw_fetch(pair, split=False):
            xwt = xw_pool.tile([128, 3072], FP16, name="xwcat", tag="xw")
            src = (xw_d[:, :, pair * 256:pair * 256 + 256]
                   .rearrange("t p n -> p t n"))
            dst = xwt.rearrange("p (t n) -> p t n", t=12)
            if split:  # fill: first tiles land early so block 0 starts fast
                nc.sync.dma_start(dst[:, 0:4], src[:, 0:4])
                nc.scalar.dma_start(dst[:, 4:12], src[:, 4:12])
            else:
                nc.sync.dma_start(dst, src)
            return xwt

        x2t_prev = [None, None, None]
        ft_q = [None, None]
        xw_tiles = {0: xw_fetch(0, split=True)}
        for it in range(n_blk + 2):
            if it % 2 == 0 and it // 2 + 1 < n_blk // 2:
                # prefetch the next pair's input one iteration ahead so it
                # never queues behind a dependent T1 transpose on SP
                xw_tiles[it // 2 + 1] = xw_fetch(it // 2 + 1)
            if it < n_blk:
                xwcat = xw_tiles[it // 2]
            x2t_cur = [None, None, None]
            x2cat = [None, None, None]
            prev = 1 <= it <= n_blk
            cur = it < n_blk
            # interleaved emission: conv1 tiles 4 at a time (one T1 part
            # each), conv2 groups as soon as their x2cat chunks exist
            fc = 2 <= it <= n_blk + 1
            if prev:
                x2cat[0] = conv2_evict(x2t_prev[0], 0)
            if cur:
                conv1_window(it, xwcat, it % 2, 0)
                conv1_window(it, xwcat, it % 2, 1)  # the R window: its two
                # 1-buffer reduces interleave with the fc matmuls below
            if fc:
                # fc chain early: the T2 transpose it consumes was issued
                # mid-way through the previous iteration, so it has landed
                psf1 = fc_front(it - 2, ft_q[it % 2])
            if cur:
                x2t_cur[0] = t1_part(it, 0)
            if prev:
                x2cat[1] = conv2_evict(x2t_prev[1], 1)
                conv2_group(it - 1, x2cat, 0)
                conv2_group(it - 1, x2cat, 1)
            if fc:
                fc_back(it - 2, psf1)
            if cur:
                conv1_window(it, xwcat, it % 2, 2)
                conv1_window(it, xwcat, it % 2, 3)
                x2t_cur[1] = t1_part(it, 1)
            if prev:
                x2cat[2] = conv2_evict(x2t_prev[2], 2)
                conv2_group(it - 1, x2cat, 2)
                conv2_group(it - 1, x2cat, 3)
                ft_q[(it - 1) % 2] = t2_issue(it - 1)
            if cur:
                conv1_window(it, xwcat, it % 2, 4)
                conv1_window(it, xwcat, it % 2, 5)
                x2t_cur[2] = t1_part(it, 2)
            x2t_prev = x2t_cur

        # ---------------- batched log_softmax epilogue ----------------
        epilogue(0, n_blk)

    nc.compile()
    return nc


_PROGRAM_CACHE = {}


def _get_program(b_core):
    if b_core not in _PROGRAM_CACHE:
        _PROGRAM_CACHE[b_core] = _build(b_core)
    return _PROGRAM_CACHE[b_core]


def make_in_maps(x, weights, b_core=B_CORE, n_cores=N_CORES):
    """Shard x over cores; replicate the (rearranged) parameters."""
    f32 = np.float32
    xr = np.asarray(x, dtype=f32).reshape(-1, 28, 28)
    in_maps = []
    for c in range(n_cores):
        xc = xr[c * b_core:(c + 1) * b_core]  # [b_core, 28, 28]
        xwin = np.empty((12, 128, b_core), np.float16)
        for w in range(6):
            for h in range(2):
                win = xc[:, 4 * w:4 * w + 8, 12 * h:12 * h + 16]
                xwin[w * 2 + h] = win.reshape(b_core, 128).T
        m = {"xw": np.ascontiguousarray(xwin)}
        m.update(weights)
        in_maps.append(m)
    return in_maps


def kernel(**inputs):
    x = np.asarray(inputs["x"], dtype=np.float32)
    weights = prep_weights(
        np.asarray(inputs["mask_w"], np.float32),
        np.asarray(inputs["conv1_w"], np.float32),
        np.asarray(inputs["conv1_b"], np.float32),
        np.asarray(inputs["conv2_w"], np.float32),
        np.asarray(inputs["conv2_b"], np.float32),
        np.asarray(inputs["fc1_w"], np.float32),
        np.asarray(inputs["fc1_b"], np.float32),
        np.asarray(inputs["fc2_w"], np.float32),
        np.asarray(inputs["fc2_b"], np.float32),
    )
    nc = _get_program(B_CORE)
    in_maps = make_in_maps(x, weights)
    res = run_bass_kernel_spmd(nc, in_maps, list(range(N_CORES)))
    out = np.concatenate([res.results[c]["y"] for c in range(N_CORES)], axis=0)
    return np.ascontiguousarray(out.astype(np.float32))


if __name__ == "__main__":
    rng = np.random.default_rng(0)
    ins = {
        "x": rng.standard_normal((B_TOTAL, 1, 28, 28), dtype=np.float32),
        "mask_w": rng.standard_normal((28, 28), dtype=np.float32) * 0.1,
        "conv1_w": rng.standard_normal((10, 1, 5, 5), dtype=np.float32) * 0.2,
        "conv1_b": rng.standard_normal((10,), dtype=np.float32) * 0.1,
        "conv2_w": rng.standard_normal((20, 10, 5, 5), dtype=np.float32) * 0.06,
        "conv2_b": rng.standard_normal((20,), dtype=np.float32) * 0.1,
        "fc1_w": rng.standard_normal((50, 320), dtype=np.float32) * 0.05,
        "fc1_b": rng.standard_normal((50,), dtype=np.float32) * 0.1,
        "fc2_w": rng.standard_normal((10, 50), dtype=np.float32) * 0.14,
        "fc2_b": rng.standard_normal((10,), dtype=np.float32) * 0.1,
    }
    out = kernel(**ins)
    print(out.shape, out.dtype, out[:2])
